# revision 25
# baseline (speedup 1.0000x reference)
"""Causal self-attention (B=4, T=2048, E=2048, H=16) on 8 trn2 NeuronCores.

Tensor-parallel over heads: 2 heads per core. Per-core Bass/Tile kernel:
  qkvT = w_qkvT.T @ xT  (fp32r matmuls), fused rotate-half RoPE (DVE),
  attention in transposed layout (scoresT = k.T@q so softmax'd probs feed
  the PV matmul directly, no transposes of P), causal block skipping with
  narrowed diagonal tiles, softmax without max-subtraction (scores are
  O(5), exp cannot overflow), denominator via a ones-row matmul
  accumulated next to PV, both heads software-pipelined (PV lags scores
  by one step so exp latency is hidden).

  Token resharding via four bf16 AllToAlls (one per batch, 1 MB/rank
  each), each fired as soon as its batch's attention finishes; o_proj
  (bf16) for that batch runs under the next batch's compute. Tokens are
  interleaved across ranks in 256-token blocks so every batch's A2A
  involves all 8 ranks symmetrically.

Host-side prep in kernel(): transpose x, permute q/k weight rows so RoPE
becomes rotate-half (scores invariant under a shared d-permutation), fold
the 1/sqrt(d) scale into w_q, precompute cos/sin tables, shard w_qkv by
head, cast w_o to bf16.
"""

import sys

sys.path.insert(0, "/opt/trn_rl_repo")

import ml_dtypes
import numpy as np

B, T, E, H = 4, 2048, 2048, 16
HD = E // H            # 128
NC_ = 8                # cores
HPC = H // NC_         # heads per core
CL = 3 * HPC * HD      # local qkv channels = 768
VOFF = 2 * HPC * 128   # column offset of v channels in wqkvT = 512
BLK = T // NC_         # token block per rank per batch = 256
TT = 512               # token tile
EB = E // 128          # 16 contraction blocks
NBT = T // TT          # 4 token tiles per batch
KB = T // 128          # 16 key blocks per batch

_BUILT = None


def _build(b_run=B):
    import concourse.mybir as mybir
    import concourse.tile as tile
    from concourse import bacc

    f32 = mybir.dt.float32
    f32r = mybir.dt.float32r
    bf16 = mybir.dt.bfloat16
    ACT = mybir.ActivationFunctionType
    MUL = mybir.AluOpType.mult

    BT = b_run * T

    nc = bacc.Bacc("TRN2", target_bir_lowering=False, debug=False,
                   num_devices=NC_)

    xT = nc.dram_tensor("xT", [E, BT], f32, kind="ExternalInput")
    wqkvT = nc.dram_tensor("wqkvT", [E, CL], f32, kind="ExternalInput")
    woT = nc.dram_tensor("woT", [E, E], bf16, kind="ExternalInput")
    cosT = nc.dram_tensor("cosT", [64, T], f32, kind="ExternalInput")
    sinT = nc.dram_tensor("sinT", [64, T], f32, kind="ExternalInput")
    trimask = nc.dram_tensor("trimask", [128, 128], f32, kind="ExternalInput")
    outT = nc.dram_tensor("outT", [E, b_run * BLK], f32, kind="ExternalOutput")

    xT_r = xT.rearrange("(eh p) t -> p eh t", p=128)
    woT_r = woT.rearrange("(cb p) e -> p cb e", p=128)

    with tile.TileContext(nc) as tc:
        with tc.tile_pool(name="consts", bufs=1) as consts, \
             tc.tile_pool(name="dram", bufs=1, space="DRAM") as dram, \
             tc.tile_pool(name="wq", bufs=1) as wq_pool, \
             tc.tile_pool(name="xt", bufs=5) as xt_pool, \
             tc.tile_pool(name="qk", bufs=1) as qk_pool, \
             tc.tile_pool(name="pt", bufs=5) as pt_pool, \
             tc.tile_pool(name="eps", bufs=1) as eps_pool, \
             tc.tile_pool(name="wo", bufs=2) as wo_pool, \
             tc.tile_pool(name="oo", bufs=2) as oo_pool, \
             tc.tile_pool(name="bps", bufs=3, space="PSUM") as bps, \
             tc.tile_pool(name="ops_o", bufs=1, space="PSUM") as ops_o, \
             tc.tile_pool(name="pps", bufs=1, space="PSUM") as pps, \
             tc.tile_pool(name="zps", bufs=1, space="PSUM") as zps:
            cos_sb = consts.tile([64, T], f32)
            sin_sb = consts.tile([64, T], f32)
            tri_sb = consts.tile([128, 128], f32)
            ones_col = consts.tile([128, 1], f32)
            ones_row = consts.tile([1, 128], f32)
            nc.sync.dma_start(out=cos_sb[:], in_=cosT[:])
            nc.sync.dma_start(out=sin_sb[:], in_=sinT[:])
            nc.sync.dma_start(out=tri_sb[:], in_=trimask[:])
            nc.vector.memset(ones_col[:], 1.0)
            nc.vector.memset(ones_row[:], 1.0)

            agl = [dram.tile([E, BLK], bf16, name=f"agl{b}")
                   for b in range(b_run)]
            agf = [dram.tile([E, BLK], bf16, name=f"agf{b}")
                   for b in range(b_run)]

            xcache = {}

            def load_xtile(b, tt):
                if (b, tt) in xcache:
                    return xcache.pop((b, tt))
                t0 = b * T + tt * TT
                xc = []
                for h in range(4):
                    xch = xt_pool.tile([128, EB // 4, TT], f32, tag="xt",
                                       name="xch")
                    nc.sync.dma_start(
                        out=xch[:].bitcast(f32r),
                        in_=xT_r[:, h * 4:(h + 1) * 4,
                                 t0:t0 + TT].bitcast(f32r))
                    xc.append(xch)
                return xc

            xcache[(0, 0)] = load_xtile(0, 0)

            w_sb = wq_pool.tile([128, EB, CL], f32)
            wqkvT_r = wqkvT.rearrange("(eb p) c -> p eb c", p=128)
            for e in range(EB):
                nc.sync.dma_start(
                    out=w_sb[:, e, :].bitcast(f32r),
                    in_=wqkvT_r[:, e, :].bitcast(f32r))

            def wv(e, cs):
                return w_sb[:, e, cs].bitcast(f32r)

            atiles = {}

            def emit_oproj(b, ebs):
                if b not in atiles:
                    a_tile = oo_pool.tile([128, EB, BLK], bf16, tag="at",
                                          name="a_tile", bufs=2)
                    nc.gpsimd.dma_start(
                        out=a_tile[:],
                        in_=agf[b].rearrange("(cb p) t -> p cb t", p=128))
                    atiles[b] = a_tile
                a_tile = atiles[b]
                for eb in ebs:
                    wo_eb = wo_pool.tile([128, EB, 128], bf16, tag="wo",
                                         name="wo_eb")
                    nc.gpsimd.dma_start(
                        out=wo_eb[:],
                        in_=woT_r[:, :, eb * 128:(eb + 1) * 128])
                    pso = pps.tile([128, BLK], f32, tag="pso", name="pso")
                    for cb in range(EB):
                        nc.tensor.matmul(
                            pso[:], wo_eb[:, cb, :], a_tile[:, cb, :],
                            start=(cb == 0), stop=(cb == EB - 1))
                    ot = oo_pool.tile([128, BLK], f32, tag="ot", name="ot",
                                      bufs=2)
                    nc.scalar.activation(ot[:], pso[:], ACT.Copy)
                    nc.gpsimd.dma_start(
                        out=outT[eb * 128:(eb + 1) * 128,
                                 b * BLK:(b + 1) * BLK],
                        in_=ot[:])

            for b in range(b_run):
                q_sb = [qk_pool.tile([HD, T], f32, tag=f"q{j}",
                                     name=f"q_sb{j}") for j in range(HPC)]
                k_sb = [qk_pool.tile([HD, T], f32, tag=f"k{j}",
                                     name=f"k_sb{j}") for j in range(HPC)]
                v_hold = qk_pool.tile([128, KB, HPC * HD], f32, tag="vh",
                                      name="v_hold")

                # ---- QKV projection for this batch ----
                for tt in range(NBT):
                    xc = load_xtile(b, tt)

                    def xv(e, ts=slice(None)):
                        return xc[e // 4][:, e % 4, ts].bitcast(f32r)

                    # q/k channels: c-blocks [q0,k0,q1,k1]
                    for c in range(2 * HPC):
                        j, is_k = c // 2, c % 2
                        ps = bps.tile([128, TT], f32, tag="big", name="ps_qk")
                        for e in range(EB):
                            nc.tensor.matmul(
                                ps[:], wv(e, slice(c * 128, (c + 1) * 128)),
                                xv(e), start=(e == 0), stop=(e == EB - 1))
                        # rotate-half rope out of PSUM
                        cs = cos_sb[:, tt * TT:(tt + 1) * TT]
                        sn = sin_sb[:, tt * TT:(tt + 1) * TT]
                        t1 = eps_pool.tile([128, TT], f32, tag="t1",
                                           name="t1", bufs=2)
                        t2 = eps_pool.tile([128, TT], f32, tag="t2",
                                           name="t2", bufs=2)
                        nc.vector.tensor_mul(t1[0:64, :], ps[0:64, :], cs)
                        nc.vector.tensor_mul(t1[64:128, :], ps[64:128, :], cs)
                        nc.vector.scalar_tensor_tensor(
                            t2[0:64, :], ps[64:128, :], -1.0, sn, MUL, MUL)
                        nc.vector.tensor_mul(t2[64:128, :], ps[0:64, :], sn)
                        dst = (k_sb if is_k else q_sb)[j]
                        nc.vector.tensor_add(
                            dst[:, tt * TT:(tt + 1) * TT].bitcast(f32r),
                            t1[:], t2[:])
                    # v channels, natural (t, d) layout, straight to SBUF
                    for tb in range(TT // 128):
                        psv = bps.tile([128, HPC * HD], f32, tag="big",
                                       name="psv")
                        for e in range(EB):
                            nc.tensor.matmul(
                                psv[:], xv(e, slice(tb * 128, (tb + 1) * 128)),
                                wv(e, slice(VOFF, CL)),
                                start=(e == 0), stop=(e == EB - 1))
                        kb = tt * (TT // 128) + tb
                        nc.scalar.activation(v_hold[:, kb, :].bitcast(f32r),
                                             psv[:], ACT.Copy)

                if b > 0:
                    emit_oproj(b - 1, range(0, EB // 2))

                # ---- attention: heads interleaved, PV lags scores ----
                for qt in range(NBT):
                    nkt = (qt + 1) * (TT // 128)
                    ps_o = [ops_o.tile([128, TT], f32, tag=f"o{j}",
                                       name=f"ps_o{j}") for j in range(HPC)]
                    ps_z = [zps.tile([1, TT], f32, tag=f"z{j}",
                                     name=f"ps_z{j}") for j in range(HPC)]
                    pts = {}
                    for kt in range(nkt + 1):
                        if kt < nkt:
                            m = kt - qt * (TT // 128)
                            lo = max(m, 0) * 128
                            for j in range(HPC):
                                ps_s = bps.tile([128, TT], f32, tag="big",
                                                name="ps_s")
                                nc.tensor.matmul(
                                    ps_s[:, lo:],
                                    k_sb[j][:, kt * 128:(kt + 1) * 128].bitcast(f32r),
                                    q_sb[j][:, qt * TT + lo:(qt + 1) * TT].bitcast(f32r),
                                    start=True, stop=True)
                                pt = pt_pool.tile([128, TT], f32, tag="pt",
                                                  name="pt")
                                nc.scalar.activation(
                                    pt[:, lo:].bitcast(f32r), ps_s[:, lo:],
                                    ACT.Exp)
                                if m >= 0:
                                    nc.vector.tensor_mul(
                                        pt[:, lo:lo + 128].bitcast(f32r),
                                        pt[:, lo:lo + 128], tri_sb[:])
                                pts[(j, kt)] = pt
                        if kt > 0:
                            pk = kt - 1
                            lo = max(pk - qt * (TT // 128), 0) * 128
                            for j in range(HPC):
                                pt = pts.pop((j, pk))
                                nc.tensor.matmul(
                                    ps_o[j][:, lo:],
                                    v_hold[:, pk, j * HD:(j + 1) * HD].bitcast(f32r),
                                    pt[:, lo:].bitcast(f32r),
                                    start=(pk == 0), stop=(pk == nkt - 1))
                                nc.tensor.matmul(
                                    ps_z[j][:, lo:],
                                    ones_col[:].bitcast(f32r),
                                    pt[:, lo:].bitcast(f32r),
                                    start=(pk == 0), stop=(pk == nkt - 1))
                    for j in range(HPC):
                        # out = ps_o * broadcast(1/Z)
                        zrow = eps_pool.tile([1, TT], f32, tag="zr",
                                             name="zrow")
                        nc.vector.tensor_copy(zrow[:], ps_z[j][:])
                        zri = eps_pool.tile([1, TT], f32, tag="zri",
                                            name="zri")
                        nc.vector.reciprocal_approx_fast(zri[:], zrow[:])
                        zb = eps_pool.tile([128, TT], f32, tag="zb",
                                           name="zb", bufs=2)
                        nc.gpsimd.partition_broadcast(zb[:], zri[:])
                        ao = eps_pool.tile([128, TT], bf16, tag="ao",
                                           name="ao", bufs=2)
                        nc.vector.tensor_mul(ao[:], ps_o[j][:], zb[:])
                        # scatter the two 256-token halves to their dest
                        # chunks: token b*T + qt*TT + i -> chunk qt*2 + i//BLK
                        for hblk in range(TT // BLK):
                            dch = qt * (TT // BLK) + hblk
                            nc.gpsimd.dma_start(
                                out=agl[b][dch * (E // NC_) + j * HD:
                                           dch * (E // NC_) + (j + 1) * HD, :],
                                in_=ao[:, hblk * BLK:(hblk + 1) * BLK])

                if b > 0:
                    emit_oproj(b - 1, range(EB // 2, EB))

                # ---- per-batch all-to-all; o_proj deferred so the PE
                # queue services the NEXT batch's QKV during the A2A ----
                nc.gpsimd.collective_compute(
                    "AllToAll", mybir.AluOpType.bypass,
                    replica_groups=[list(range(NC_))],
                    ins=[agl[b][:]], outs=[agf[b][:]])

            emit_oproj(b_run - 1, range(EB))
    nc.compile()
    return nc


def _prep_inputs(x, freqs, w_qkv, w_o, b_run=B):
    xf = np.ascontiguousarray(x, dtype=np.float32).reshape(b_run * T, E)
    xT = np.ascontiguousarray(xf.T)

    wq = w_qkv[0:E].reshape(H, HD, E)
    wk = w_qkv[E:2 * E].reshape(H, HD, E)
    wvv = w_qkv[2 * E:3 * E].reshape(H, HD, E)
    perm = np.concatenate([np.arange(0, HD, 2), np.arange(1, HD, 2)])
    scale = np.float32(1.0 / np.sqrt(HD))
    wq_p = wq[:, perm, :] * scale
    wk_p = wk[:, perm, :]

    cos = np.cos(freqs.astype(np.float32))
    sin = np.sin(freqs.astype(np.float32))
    cosT = np.ascontiguousarray(cos.T)
    sinT = np.ascontiguousarray(sin.T)
    tri = (np.arange(128)[:, None] <= np.arange(128)[None, :]).astype(np.float32)
    w_oT = np.ascontiguousarray(w_o.T.astype(ml_dtypes.bfloat16))

    in_maps = []
    for r in range(NC_):
        blocks = []
        for j in range(HPC):
            h = r * HPC + j
            blocks += [wq_p[h].T, wk_p[h].T]
        blocks += [wvv[r * HPC + j].T for j in range(HPC)]
        wqkvT_loc = np.ascontiguousarray(
            np.concatenate(blocks, axis=1), dtype=np.float32)
        in_maps.append({
            "xT": xT,
            "wqkvT": wqkvT_loc,
            "woT": w_oT,
            "cosT": cosT,
            "sinT": sinT,
            "trimask": tri,
        })
    return in_maps


def kernel(x, freqs, w_qkv, w_o, _trace=False, _b_run=B):
    global _BUILT
    from concourse.bass_utils import run_bass_kernel_spmd

    if _BUILT is None or _BUILT[1] != _b_run:
        _BUILT = (_build(_b_run), _b_run)
    nc = _BUILT[0]

    in_maps = _prep_inputs(np.asarray(x), np.asarray(freqs),
                           np.asarray(w_qkv), np.asarray(w_o), _b_run)
    res = run_bass_kernel_spmd(nc, in_maps, core_ids=list(range(NC_)),
                               trace=_trace)
    # core r owns tokens b*T + r*BLK + [0, BLK) for each batch b
    out = np.empty((E, _b_run * T), np.float32)
    for r in range(NC_):
        o = res.results[r]["outT"]
        for b in range(_b_run):
            out[:, b * T + r * BLK:b * T + (r + 1) * BLK] = \
                o[:, b * BLK:(b + 1) * BLK]
    out = np.ascontiguousarray(out.T).reshape(_b_run, T, E)
    if _trace:
        kernel.last_results = res
    return out.astype(np.float32, copy=False)


# revision 26
# speedup vs baseline: 1.0019x; 1.0019x over previous
"""Causal self-attention (B=4, T=2048, E=2048, H=16) on 8 trn2 NeuronCores.

Tensor-parallel over heads: 2 heads per core. Per-core Bass/Tile kernel:
  qkvT = w_qkvT.T @ xT  (fp32r matmuls), fused rotate-half RoPE (DVE),
  attention in transposed layout (scoresT = k.T@q so softmax'd probs feed
  the PV matmul directly, no transposes of P), causal block skipping with
  narrowed diagonal tiles, softmax without max-subtraction (scores are
  O(5), exp cannot overflow), denominator via a ones-row matmul
  accumulated next to PV, both heads software-pipelined (PV lags scores
  by one step so exp latency is hidden).

  Token resharding via four bf16 AllToAlls (one per batch, 1 MB/rank
  each), each fired as soon as its batch's attention finishes; o_proj
  (bf16) for that batch runs under the next batch's compute. Tokens are
  interleaved across ranks in 256-token blocks so every batch's A2A
  involves all 8 ranks symmetrically.

Host-side prep in kernel(): transpose x, permute q/k weight rows so RoPE
becomes rotate-half (scores invariant under a shared d-permutation), fold
the 1/sqrt(d) scale into w_q, precompute cos/sin tables, shard w_qkv by
head, cast w_o to bf16.
"""

import sys

sys.path.insert(0, "/opt/trn_rl_repo")

import ml_dtypes
import numpy as np

B, T, E, H = 4, 2048, 2048, 16
HD = E // H            # 128
NC_ = 8                # cores
HPC = H // NC_         # heads per core
CL = 3 * HPC * HD      # local qkv channels = 768
VOFF = 2 * HPC * 128   # column offset of v channels in wqkvT = 512
BLK = T // NC_         # token block per rank per batch = 256
TT = 512               # token tile
EB = E // 128          # 16 contraction blocks
NBT = T // TT          # 4 token tiles per batch
KB = T // 128          # 16 key blocks per batch

_BUILT = None


def _build(b_run=B):
    import concourse.mybir as mybir
    import concourse.tile as tile
    from concourse import bacc

    f32 = mybir.dt.float32
    f32r = mybir.dt.float32r
    bf16 = mybir.dt.bfloat16
    ACT = mybir.ActivationFunctionType
    MUL = mybir.AluOpType.mult

    BT = b_run * T

    nc = bacc.Bacc("TRN2", target_bir_lowering=False, debug=False,
                   num_devices=NC_)

    xT = nc.dram_tensor("xT", [E, BT], f32, kind="ExternalInput")
    wqkvT = nc.dram_tensor("wqkvT", [E, CL], f32, kind="ExternalInput")
    woT = nc.dram_tensor("woT", [E, E], bf16, kind="ExternalInput")
    cosT = nc.dram_tensor("cosT", [64, T], f32, kind="ExternalInput")
    sinT = nc.dram_tensor("sinT", [64, T], f32, kind="ExternalInput")
    trimask = nc.dram_tensor("trimask", [128, 128], f32, kind="ExternalInput")
    outT = nc.dram_tensor("outT", [E, b_run * BLK], f32, kind="ExternalOutput")

    xT_r = xT.rearrange("(eh p) t -> p eh t", p=128)
    woT_r = woT.rearrange("(cb p) e -> p cb e", p=128)

    with tile.TileContext(nc) as tc:
        with tc.tile_pool(name="consts", bufs=1) as consts, \
             tc.tile_pool(name="dram", bufs=1, space="DRAM") as dram, \
             tc.tile_pool(name="wq", bufs=1) as wq_pool, \
             tc.tile_pool(name="xt", bufs=5) as xt_pool, \
             tc.tile_pool(name="qk", bufs=1) as qk_pool, \
             tc.tile_pool(name="pt", bufs=4) as pt_pool, \
             tc.tile_pool(name="eps", bufs=1) as eps_pool, \
             tc.tile_pool(name="wo", bufs=2) as wo_pool, \
             tc.tile_pool(name="oo", bufs=2) as oo_pool, \
             tc.tile_pool(name="bps", bufs=3, space="PSUM") as bps, \
             tc.tile_pool(name="ops_o", bufs=1, space="PSUM") as ops_o, \
             tc.tile_pool(name="pps", bufs=1, space="PSUM") as pps, \
             tc.tile_pool(name="zps", bufs=1, space="PSUM") as zps:
            cos_sb = consts.tile([64, T], f32)
            sin_sb = consts.tile([64, T], f32)
            tri_sb = consts.tile([128, 128], f32)
            ones_col = consts.tile([128, 1], f32)
            ones_row = consts.tile([1, 128], f32)
            nc.sync.dma_start(out=cos_sb[:], in_=cosT[:])
            nc.sync.dma_start(out=sin_sb[:], in_=sinT[:])
            nc.sync.dma_start(out=tri_sb[:], in_=trimask[:])
            nc.vector.memset(ones_col[:], 1.0)
            nc.vector.memset(ones_row[:], 1.0)

            agl = [dram.tile([E, BLK], bf16, name=f"agl{b}")
                   for b in range(b_run)]
            agf = [dram.tile([E, BLK], bf16, name=f"agf{b}")
                   for b in range(b_run)]

            xcache = {}

            def load_xtile(b, tt):
                if (b, tt) in xcache:
                    return xcache.pop((b, tt))
                t0 = b * T + tt * TT
                xc = []
                for h in range(4):
                    xch = xt_pool.tile([128, EB // 4, TT], f32, tag="xt",
                                       name="xch")
                    nc.sync.dma_start(
                        out=xch[:].bitcast(f32r),
                        in_=xT_r[:, h * 4:(h + 1) * 4,
                                 t0:t0 + TT].bitcast(f32r))
                    xc.append(xch)
                return xc

            xcache[(0, 0)] = load_xtile(0, 0)

            w_sb = wq_pool.tile([128, EB, CL], f32)
            wqkvT_r = wqkvT.rearrange("(eb p) c -> p eb c", p=128)
            for e in range(EB):
                nc.sync.dma_start(
                    out=w_sb[:, e, :].bitcast(f32r),
                    in_=wqkvT_r[:, e, :].bitcast(f32r))

            def wv(e, cs):
                return w_sb[:, e, cs].bitcast(f32r)

            atiles = {}

            def emit_oproj(b, ebs):
                if b not in atiles:
                    a_tile = oo_pool.tile([128, EB, BLK], bf16, tag="at",
                                          name="a_tile", bufs=2)
                    nc.gpsimd.dma_start(
                        out=a_tile[:],
                        in_=agf[b].rearrange("(cb p) t -> p cb t", p=128))
                    atiles[b] = a_tile
                a_tile = atiles[b]
                for eb in ebs:
                    wo_eb = wo_pool.tile([128, EB, 128], bf16, tag="wo",
                                         name="wo_eb")
                    nc.gpsimd.dma_start(
                        out=wo_eb[:],
                        in_=woT_r[:, :, eb * 128:(eb + 1) * 128])
                    pso = pps.tile([128, BLK], f32, tag="pso", name="pso")
                    for cb in range(EB):
                        nc.tensor.matmul(
                            pso[:], wo_eb[:, cb, :], a_tile[:, cb, :],
                            start=(cb == 0), stop=(cb == EB - 1))
                    ot = oo_pool.tile([128, BLK], f32, tag="ot", name="ot",
                                      bufs=2)
                    nc.scalar.activation(ot[:], pso[:], ACT.Copy)
                    nc.gpsimd.dma_start(
                        out=outT[eb * 128:(eb + 1) * 128,
                                 b * BLK:(b + 1) * BLK],
                        in_=ot[:])

            for b in range(b_run):
                q_sb = [qk_pool.tile([HD, T], f32, tag=f"q{j}",
                                     name=f"q_sb{j}") for j in range(HPC)]
                k_sb = [qk_pool.tile([HD, T], f32, tag=f"k{j}",
                                     name=f"k_sb{j}") for j in range(HPC)]
                v_hold = qk_pool.tile([128, KB, HPC * HD], f32, tag="vh",
                                      name="v_hold")

                # ---- QKV projection for this batch ----
                for tt in range(NBT):
                    xc = load_xtile(b, tt)

                    def xv(e, ts=slice(None)):
                        return xc[e // 4][:, e % 4, ts].bitcast(f32r)

                    # q/k channels: c-blocks [q0,k0,q1,k1]
                    for c in range(2 * HPC):
                        j, is_k = c // 2, c % 2
                        ps = bps.tile([128, TT], f32, tag="big", name="ps_qk")
                        for e in range(EB):
                            nc.tensor.matmul(
                                ps[:], wv(e, slice(c * 128, (c + 1) * 128)),
                                xv(e), start=(e == 0), stop=(e == EB - 1))
                        # rotate-half rope out of PSUM
                        cs = cos_sb[:, tt * TT:(tt + 1) * TT]
                        sn = sin_sb[:, tt * TT:(tt + 1) * TT]
                        t1 = eps_pool.tile([128, TT], f32, tag="t1",
                                           name="t1", bufs=2)
                        t2 = eps_pool.tile([128, TT], f32, tag="t2",
                                           name="t2", bufs=2)
                        nc.vector.tensor_mul(t1[0:64, :], ps[0:64, :], cs)
                        nc.vector.tensor_mul(t1[64:128, :], ps[64:128, :], cs)
                        nc.vector.scalar_tensor_tensor(
                            t2[0:64, :], ps[64:128, :], -1.0, sn, MUL, MUL)
                        nc.vector.tensor_mul(t2[64:128, :], ps[0:64, :], sn)
                        dst = (k_sb if is_k else q_sb)[j]
                        nc.vector.tensor_add(
                            dst[:, tt * TT:(tt + 1) * TT].bitcast(f32r),
                            t1[:], t2[:])
                    # v channels, natural (t, d) layout, straight to SBUF
                    for tb in range(TT // 128):
                        psv = bps.tile([128, HPC * HD], f32, tag="big",
                                       name="psv")
                        for e in range(EB):
                            nc.tensor.matmul(
                                psv[:], xv(e, slice(tb * 128, (tb + 1) * 128)),
                                wv(e, slice(VOFF, CL)),
                                start=(e == 0), stop=(e == EB - 1))
                        kb = tt * (TT // 128) + tb
                        nc.scalar.activation(v_hold[:, kb, :].bitcast(f32r),
                                             psv[:], ACT.Copy)

                if b > 0:
                    emit_oproj(b - 1, range(0, EB // 2))

                # ---- attention: heads interleaved, PV lags scores ----
                for qt in range(NBT):
                    nkt = (qt + 1) * (TT // 128)
                    ps_o = [ops_o.tile([128, TT], f32, tag=f"o{j}",
                                       name=f"ps_o{j}") for j in range(HPC)]
                    ps_z = [zps.tile([1, TT], f32, tag=f"z{j}",
                                     name=f"ps_z{j}") for j in range(HPC)]
                    pts = {}
                    for kt in range(nkt + 1):
                        if kt < nkt:
                            m = kt - qt * (TT // 128)
                            lo = max(m, 0) * 128
                            for j in range(HPC):
                                ps_s = bps.tile([128, TT], f32, tag="big",
                                                name="ps_s")
                                nc.tensor.matmul(
                                    ps_s[:, lo:],
                                    k_sb[j][:, kt * 128:(kt + 1) * 128].bitcast(f32r),
                                    q_sb[j][:, qt * TT + lo:(qt + 1) * TT].bitcast(f32r),
                                    start=True, stop=True)
                                pt = pt_pool.tile([128, TT], f32, tag="pt",
                                                  name="pt")
                                nc.scalar.activation(
                                    pt[:, lo:].bitcast(f32r), ps_s[:, lo:],
                                    ACT.Exp)
                                if m >= 0:
                                    nc.vector.tensor_mul(
                                        pt[:, lo:lo + 128].bitcast(f32r),
                                        pt[:, lo:lo + 128], tri_sb[:])
                                pts[(j, kt)] = pt
                        if kt > 0:
                            pk = kt - 1
                            lo = max(pk - qt * (TT // 128), 0) * 128
                            for j in range(HPC):
                                pt = pts.pop((j, pk))
                                nc.tensor.matmul(
                                    ps_o[j][:, lo:],
                                    v_hold[:, pk, j * HD:(j + 1) * HD].bitcast(f32r),
                                    pt[:, lo:].bitcast(f32r),
                                    start=(pk == 0), stop=(pk == nkt - 1))
                                nc.tensor.matmul(
                                    ps_z[j][:, lo:],
                                    ones_col[:].bitcast(f32r),
                                    pt[:, lo:].bitcast(f32r),
                                    start=(pk == 0), stop=(pk == nkt - 1))
                    for j in range(HPC):
                        # out = ps_o * broadcast(1/Z)
                        zrow = eps_pool.tile([1, TT], f32, tag="zr",
                                             name="zrow")
                        nc.vector.tensor_copy(zrow[:], ps_z[j][:])
                        zri = eps_pool.tile([1, TT], f32, tag="zri",
                                            name="zri")
                        nc.vector.reciprocal_approx_fast(zri[:], zrow[:])
                        zb = eps_pool.tile([128, TT], f32, tag="zb",
                                           name="zb", bufs=2)
                        nc.gpsimd.partition_broadcast(zb[:], zri[:])
                        ao = eps_pool.tile([128, TT], bf16, tag="ao",
                                           name="ao", bufs=2)
                        nc.vector.tensor_mul(ao[:], ps_o[j][:], zb[:])
                        # scatter the two 256-token halves to their dest
                        # chunks: token b*T + qt*TT + i -> chunk qt*2 + i//BLK
                        for hblk in range(TT // BLK):
                            dch = qt * (TT // BLK) + hblk
                            nc.gpsimd.dma_start(
                                out=agl[b][dch * (E // NC_) + j * HD:
                                           dch * (E // NC_) + (j + 1) * HD, :],
                                in_=ao[:, hblk * BLK:(hblk + 1) * BLK])

                if b > 0:
                    emit_oproj(b - 1, range(EB // 2, EB))

                # ---- per-batch all-to-all; o_proj deferred so the PE
                # queue services the NEXT batch's QKV during the A2A ----
                nc.gpsimd.collective_compute(
                    "AllToAll", mybir.AluOpType.bypass,
                    replica_groups=[list(range(NC_))],
                    ins=[agl[b][:]], outs=[agf[b][:]])

            emit_oproj(b_run - 1, range(EB))
    nc.compile()
    return nc


def _prep_inputs(x, freqs, w_qkv, w_o, b_run=B):
    xf = np.ascontiguousarray(x, dtype=np.float32).reshape(b_run * T, E)
    xT = np.ascontiguousarray(xf.T)

    wq = w_qkv[0:E].reshape(H, HD, E)
    wk = w_qkv[E:2 * E].reshape(H, HD, E)
    wvv = w_qkv[2 * E:3 * E].reshape(H, HD, E)
    perm = np.concatenate([np.arange(0, HD, 2), np.arange(1, HD, 2)])
    scale = np.float32(1.0 / np.sqrt(HD))
    wq_p = wq[:, perm, :] * scale
    wk_p = wk[:, perm, :]

    cos = np.cos(freqs.astype(np.float32))
    sin = np.sin(freqs.astype(np.float32))
    cosT = np.ascontiguousarray(cos.T)
    sinT = np.ascontiguousarray(sin.T)
    tri = (np.arange(128)[:, None] <= np.arange(128)[None, :]).astype(np.float32)
    w_oT = np.ascontiguousarray(w_o.T.astype(ml_dtypes.bfloat16))

    in_maps = []
    for r in range(NC_):
        blocks = []
        for j in range(HPC):
            h = r * HPC + j
            blocks += [wq_p[h].T, wk_p[h].T]
        blocks += [wvv[r * HPC + j].T for j in range(HPC)]
        wqkvT_loc = np.ascontiguousarray(
            np.concatenate(blocks, axis=1), dtype=np.float32)
        in_maps.append({
            "xT": xT,
            "wqkvT": wqkvT_loc,
            "woT": w_oT,
            "cosT": cosT,
            "sinT": sinT,
            "trimask": tri,
        })
    return in_maps


def kernel(x, freqs, w_qkv, w_o, _trace=False, _b_run=B):
    global _BUILT
    from concourse.bass_utils import run_bass_kernel_spmd

    if _BUILT is None or _BUILT[1] != _b_run:
        _BUILT = (_build(_b_run), _b_run)
    nc = _BUILT[0]

    in_maps = _prep_inputs(np.asarray(x), np.asarray(freqs),
                           np.asarray(w_qkv), np.asarray(w_o), _b_run)
    res = run_bass_kernel_spmd(nc, in_maps, core_ids=list(range(NC_)),
                               trace=_trace)
    # core r owns tokens b*T + r*BLK + [0, BLK) for each batch b
    out = np.empty((E, _b_run * T), np.float32)
    for r in range(NC_):
        o = res.results[r]["outT"]
        for b in range(_b_run):
            out[:, b * T + r * BLK:b * T + (r + 1) * BLK] = \
                o[:, b * BLK:(b + 1) * BLK]
    out = np.ascontiguousarray(out.T).reshape(_b_run, T, E)
    if _trace:
        kernel.last_results = res
    return out.astype(np.float32, copy=False)
